# revision 16
# baseline (speedup 1.0000x reference)
"""Trainium2 Bass kernel for a 2-layer GCN (GCNConv+BN+ReLU, GCNConv+BN).

Self-contained: kernel(**inputs) takes the full unsharded inputs, shards
destinations across 8 NeuronCores (edges partitioned by destination, one-hot
matmul scatter-add on the TensorEngine, dma_gather message fetch from fp16
feature tables, piece-wise AllGather for tables, AllReduce for BN stats),
and returns the full [100000, 128] float32 output.
"""
import numpy as np


def _apply_tile_patch():
    """This walrus build allows only 1 sync wait per CTRL instruction and
    rejects long semaphore range clears; spread Tile's exit-drain waits
    across nops and chunk the sem recycles."""
    import concourse.tile as tile_mod
    import concourse.mybir as mybir
    from concourse.vector_clock import ScopedClock

    MAXW = 1

    def _patched_drain_and_barrier(self, tick_clock, wait_clock):
        nc = self.nc
        tmp = nc.sync.nop(nofuse=True, hint="drain_wait_spread")
        wait_clock.add_sem_waits(tmp.ins, ScopedClock({None: tick_clock.global_clock}))
        waits = list(tmp.ins.sync_info.on_wait or [])
        if len(waits) > MAXW:
            tmp.ins.sync_info = mybir.SyncInfo(on_wait=waits[:MAXW], on_update=[])
            for i in range(MAXW, len(waits), MAXW):
                extra = nc.sync.nop(nofuse=True, hint="drain_wait_spread")
                extra.ins.sync_info = mybir.SyncInfo(
                    on_wait=waits[i:i + MAXW], on_update=[])
        nc.sync.drain()
        nc.all_engine_barrier()
        assert self.sems is not None
        popped = nc._tile_sem_poison_stack.pop()
        assert popped is self._sem_poison
        sems = list(self.sems.allocated().values())
        for i in range(0, len(sems), 8):
            nc.clear_and_free_semaphores(sems[i:i + 8])
        nc.all_engine_barrier()

    tile_mod.TileContext._drain_and_barrier = _patched_drain_and_barrier






from dataclasses import dataclass, field


@dataclass
class Cfg:
    N: int = 100000
    D: int = 128
    CORES: int = 8
    WIN: int = 128
    SBW: int = 6          # windows per superblock (PSUM-resident agg tiles)
    SEG: int = 25000      # gather table segment rows (int16 index reach)
    BN_EPS: float = 1e-5
    FP16: bool = True     # fp16 feature tables / messages / one-hots
    PIECES: bool = True   # piece-wise pipelined AllGather

    @property
    def PIECE(self):
        # per-core rows contributed to one pipelined AllGather piece
        return self.SEG // self.CORES

    @property
    def NPC(self):
        return self.N // self.CORES

    @property
    def NW(self):
        return (self.NPC + self.WIN - 1) // self.WIN

    @property
    def NSB(self):
        return (self.NW + self.SBW - 1) // self.SBW

    @property
    def NSEG(self):
        return (self.N + self.SEG - 1) // self.SEG


@dataclass
class Sched:
    # ops[sb][seg] = list of (w, chunk_local, part_off, cap, sv_col_local):
    #   one scatter-matmul per op. Full chunks have part_off=0, cap=128.
    #   Remainder chunks pack several windows' tails; each window-use is an
    #   op with its own slotval column (foreign partitions have val=0).
    ops: list = field(default_factory=list)
    # full[sb][seg][w] = (first chunk col local, n full chunks)
    layout: list = field(default_factory=list)
    chunk_off: dict = field(default_factory=dict)   # (sb, seg) -> first chunk col
    sv_off: dict = field(default_factory=dict)      # (sb, seg) -> first sv col
    n_chunks_call: dict = field(default_factory=dict)  # (sb, seg) -> chunks in call
    n_sv_call: dict = field(default_factory=dict)   # (sb, seg) -> sv cols in call
    CT: int = 0                                     # total chunk columns
    SVC: int = 0                                    # total slotval columns
    win_total: dict = field(default_factory=dict)   # w -> total ops (all segs)


def make_schedule(counts_max, cfg: Cfg) -> Sched:
    """counts_max[w, s] = max edge count over cores for (window w, segment s)."""
    s = Sched()
    ct = 0
    svc = 0
    for sb in range(cfg.NSB):
        w0, w1 = sb * cfg.SBW, min((sb + 1) * cfg.SBW, cfg.NW)
        per_seg_ops = []
        per_seg_layout = []
        for seg in range(cfg.NSEG):
            ops = []
            lay = {}
            nch = 0
            # full chunks per window
            rems = []
            for w in range(w0, w1):
                c = int(counts_max[w, seg])
                full = c // 128
                if full:
                    lay[w] = (nch, full)
                    for k in range(full):
                        ops.append((w, nch + k, 0, 128, None))
                    nch += full
                if c % 128:
                    rems.append((w, c % 128))
            # first-fit decreasing pack of remainders into shared chunks
            rems.sort(key=lambda t: -t[1])
            bins = []  # (used, chunk_idx, list of (w, off, cnt))
            for w, r in rems:
                placed = False
                for b in bins:
                    if b[0] + r <= 128:
                        b[2].append((w, b[0], r))
                        b[0] += r
                        placed = True
                        break
                if not placed:
                    bins.append([r, nch + len(bins), [(w, 0, r)]])
            for used, ci, uses in bins:
                for (w, off, r) in uses:
                    ops.append((w, ci, off, r, None))
            nch += len(bins)
            # assign sv columns in op order
            ops2 = []
            for (w, ci, off, cap, _) in ops:
                ops2.append((w, ci, off, cap, len(ops2)))
                s.win_total[w] = s.win_total.get(w, 0) + 1
            s.chunk_off[(sb, seg)] = ct
            s.sv_off[(sb, seg)] = svc
            s.n_chunks_call[(sb, seg)] = nch
            s.n_sv_call[(sb, seg)] = len(ops2)
            ct += nch
            svc += len(ops2)
            per_seg_ops.append(ops2)
            per_seg_layout.append(lay)
        s.ops.append(per_seg_ops)
        s.layout.append(per_seg_layout)
    s.CT = ct
    s.SVC = svc
    return s


def prep(edge_index: np.ndarray, edge_weight: np.ndarray, cfg: Cfg):
    """Host preprocessing: normalization, dest-sharding, chunk packing.

    Returns (sched, per_core) where per_core[i] = dict with idx16 [128, 8*CT]
    int16 and slotval [128, 2*CT] f32.
    """
    N = cfg.N
    row = np.concatenate([edge_index[0], np.arange(N, dtype=np.int64)]).astype(np.int64)
    col = np.concatenate([edge_index[1], np.arange(N, dtype=np.int64)]).astype(np.int64)
    w = np.concatenate([edge_weight.astype(np.float64), np.ones(N)])

    deg = np.bincount(col, weights=w, minlength=N)
    dinv = np.where(deg > 0, 1.0 / np.sqrt(np.maximum(deg, 1e-12)), 0.0)
    norm = (dinv[row] * w * dinv[col]).astype(np.float32)

    core = col // cfg.NPC
    d = col % cfg.NPC
    win = d // cfg.WIN
    slot = d % cfg.WIN
    if cfg.PIECES:
        # table row of node n (piece-wise AllGather layout):
        #   c = n // NPC, r = n % NPC, q = r // PIECE
        #   trow = q * SEG + c * PIECE + r % PIECE ; seg = q
        src_c = row // cfg.NPC
        src_r = row % cfg.NPC
        seg = src_r // cfg.PIECE
        srcid = (src_c * cfg.PIECE + src_r % cfg.PIECE).astype(np.int16)
    else:
        seg = row // cfg.SEG
        srcid = (row % cfg.SEG).astype(np.int16)

    # group id per edge: (core, sb, seg, win)
    sb = win // cfg.SBW
    NW, NSEG = cfg.NW, cfg.NSEG
    gid = ((core * cfg.NSB + sb) * NSEG + seg) * NW + win
    order = np.argsort(gid, kind="stable")
    gid_s = gid[order]
    srcid_s = srcid[order]
    slot_s = slot[order].astype(np.float32)
    norm_s = norm[order]

    # counts per (core, win, seg)
    n_gids = cfg.CORES * cfg.NSB * NSEG * NW
    cnt = np.bincount(gid, minlength=n_gids)
    cntr = cnt.reshape(cfg.CORES, cfg.NSB, NSEG, NW)
    w_idx = np.arange(NW)
    # counts[c, w, s]: only the (sb = w // SBW) plane is populated
    counts = cntr[:, w_idx // cfg.SBW, :, w_idx]      # [NW, CORES, NSEG]
    counts = counts.transpose(1, 0, 2)                # [CORES, NW, NSEG]
    counts_max = counts.max(axis=0)  # [NW, NSEG]

    sched = make_schedule(counts_max, cfg)
    CT = sched.CT
    SVC = sched.SVC

    starts = np.zeros(n_gids + 1, dtype=np.int64)
    np.cumsum(cnt, out=starts[1:])

    per_core = []
    for ci in range(cfg.CORES):
        src_p = np.zeros((CT, 128), dtype=np.int16)
        slot_p = np.zeros((SVC, 128), dtype=np.float32)
        val_p = np.zeros((SVC, 128), dtype=np.float32)
        for sbi in range(cfg.NSB):
            for sg in range(NSEG):
                cbase = sched.chunk_off[(sbi, sg)]
                svbase = sched.sv_off[(sbi, sg)]
                for (wg, ch, off, cap, svl) in sched.ops[sbi][sg]:
                    g = ((ci * cfg.NSB + sbi) * NSEG + sg) * NW + wg
                    a, b = starts[g], starts[g + 1]
                    n = b - a
                    # op order per window follows chunk order: figure out
                    # which slice of this core's edges belongs to this op
                    lay = sched.layout[sbi][sg].get(wg)
                    if lay is not None:
                        c0, nfull = lay
                        if ch < c0 + nfull:
                            # full chunk k of window wg
                            k = ch - c0
                            a2 = a + min(n, k * 128)
                            b2 = a + min(n, (k + 1) * 128)
                        else:
                            a2 = a + min(n, nfull * 128)
                            b2 = b
                    else:
                        a2, b2 = a, b
                    m = b2 - a2
                    assert m <= cap
                    if m <= 0:
                        continue
                    fl = slice((cbase + ch) * 128 + off,
                               (cbase + ch) * 128 + off + m)
                    src_p.reshape(-1)[fl] = srcid_s[a2:b2]
                    svfl = slice((svbase + svl) * 128 + off,
                                 (svbase + svl) * 128 + off + m)
                    slot_p.reshape(-1)[svfl] = slot_s[a2:b2]
                    val_p.reshape(-1)[svfl] = norm_s[a2:b2]
        # meta [128, 8*CT] int16: per (sb,seg) call gather idxs at col 8*off,
        # flat idx j -> [j % 16, j // 16]
        meta = np.zeros((128, 8 * CT), dtype=np.int16)
        for sbi in range(cfg.NSB):
            for sg in range(NSEG):
                nch = sched.n_chunks_call[(sbi, sg)]
                if not nch:
                    continue
                o = sched.chunk_off[(sbi, sg)]
                flat = src_p.reshape(-1)[o * 128:(o + nch) * 128]
                wrapped = flat.reshape(-1, 16).T  # [16, nch*8]
                meta[:, o * 8:(o + nch) * 8] = np.tile(wrapped, (8, 1))
        sv = np.stack([slot_p, val_p], axis=-1)  # [SVC, 128, 2]
        slotval = np.ascontiguousarray(
            sv.transpose(1, 0, 2).reshape(128, 2 * SVC))
        per_core.append({"meta": meta, "slotval": slotval})
    return sched, per_core


def build(nc, tc, cfg: Cfg, sched: Sched, tensors):
    """Emit the kernel into TileContext tc. tensors: dict of dram handles."""
    import concourse.mybir as mybir
    from concourse.bass import ts as _ts  # noqa: F401

    f32 = mybir.dt.float32
    f16 = mybir.dt.float16
    TDT = f16 if cfg.FP16 else f32
    i16 = mybir.dt.int16
    i32 = mybir.dt.int32
    Alu = mybir.AluOpType
    Act = mybir.ActivationFunctionType

    N, D, NPC, WIN, NW, NSB, SBW = (
        cfg.N, cfg.D, cfg.NPC, cfg.WIN, cfg.NW, cfg.NSB, cfg.SBW)
    NSEG, SEG = cfg.NSEG, cfg.SEG
    NWPAD = NW * WIN

    x = tensors["x_shard"]
    meta = tensors["meta"]
    slotval = tensors["slotval"]
    W1, W2 = tensors["W1"], tensors["W2"]
    bias = {1: tensors["b1"], 2: tensors["b2"]}
    gam = {1: tensors["g1"], 2: tensors["g2"]}
    bet = {1: tensors["be1"], 2: tensors["be2"]}
    y = tensors["y"]

    CHMAX = max(sched.n_chunks_call.values())
    SVMAX = max(sched.n_sv_call.values())

    from contextlib import ExitStack
    es = tc._gnn_exitstack = ExitStack()
    const = es.enter_context(tc.tile_pool(name="const", bufs=1))
    zpool = es.enter_context(tc.tile_pool(name="zres", bufs=1))
    spool = es.enter_context(tc.tile_pool(name="stats", bufs=1))
    work = es.enter_context(tc.tile_pool(name="work", bufs=3))
    ohp = es.enter_context(tc.tile_pool(name="oh", bufs=12))
    msgp = es.enter_context(tc.tile_pool(name="msg", bufs=3))
    idxp = es.enter_context(tc.tile_pool(name="idx", bufs=3))
    svp = es.enter_context(tc.tile_pool(name="sv", bufs=3))
    scr = es.enter_context(tc.tile_pool(name="scr", bufs=2))
    colp = es.enter_context(tc.tile_pool(name="col", bufs=1))
    psum_agg = es.enter_context(tc.tile_pool(name="pagg", bufs=SBW, space="PSUM"))
    psum_misc = es.enter_context(tc.tile_pool(name="pmisc", bufs=2, space="PSUM"))
    dram = es.enter_context(tc.tile_pool(name="dram", bufs=1, space="DRAM"))

    # ---- constants (iota rows/col provided by host via "consts" input) ----
    from concourse import library_config
    from concourse.bass import _add_dep_helper
    lib_inst = nc.gpsimd.load_library(library_config.mlp)
    _nreg_cache = {}

    def nidx_reg(v):
        if v not in _nreg_cache:
            r = nc.gpsimd.alloc_register(f"nidx_{v}")
            nc.gpsimd.reg_mov(r, v)
            _nreg_cache[v] = r
        return _nreg_cache[v]
    consts_t = const.tile([128, 129], f32, name="consts_t")
    nc.sync.dma_start(consts_t[:], tensors["consts"][:, :])
    iota_row = consts_t[:, 0:128]
    iota_col = consts_t[:, 128:129]
    ident = const.tile([128, 128], f32)
    nc.vector.tensor_scalar(
        out=ident[:], in0=iota_row, scalar1=iota_col, scalar2=None,
        op0=Alu.is_equal)
    iota_row_h = const.tile([128, 128], f16)
    nc.vector.tensor_copy(out=iota_row_h[:], in_=iota_row)

    # per-channel params as [128,1] columns
    cols = {}
    for nm in ("b1", "g1", "be1", "b2", "g2", "be2"):
        t = colp.tile([128, 1], f32, tag=nm)
        nc.sync.dma_start(t[:], tensors[nm][:, :])
        cols[nm] = t

    # ---- DRAM scratch ----
    h_local = dram.tile([NPC, D], TDT, tag="h_local")
    import os as _os_sh
    sh = _os_sh.environ.get("GNN_AG_SPACE", "Shared")
    h1_full = nc.dram_tensor("h1_full", [N, D], TDT, kind="Internal",
                             addr_space=sh)
    h2_full = nc.dram_tensor("h2_full", [N, D], TDT, kind="Internal",
                             addr_space=sh)
    h2_local = dram.tile([NPC, D], TDT, tag="h2_local")
    bn_in = dram.tile([128, 2], f32, tag="bn_in")
    bn_out = dram.tile([2, 128, 2], f32, tag="bn_out")

    import os as _os0
    if _os0.environ.get("GNN_STAGE") == "w":
        return
    # ---- phase A: h1_local = x_shard @ W1, node-major ----
    w1t = const.tile([128, 128], f32, tag="w1")
    nc.sync.dma_start(w1t[:], W1[:, :])
    w2t = const.tile([128, 128], f32, tag="w2")
    nc.sync.dma_start(w2t[:], W2[:, :])

    for t0 in range(0, NPC, 128):
        p = min(128, NPC - t0)
        xt = work.tile([128, 128], f32, tag="xt")
        nc.sync.dma_start(xt[:p, :], x[t0:t0 + p, :])
        xT = psum_misc.tile([128, 512], f32, tag="pm")
        nc.tensor.transpose(xT[:, :p], xt[:p, :], ident[:p, :p])
        xTs = work.tile([128, 128], f32, tag="xts")
        nc.vector.tensor_copy(out=xTs[:, :p], in_=xT[:, :p])
        hp = psum_misc.tile([128, 512], f32, tag="pm")
        nc.tensor.matmul(hp[:p, :128], lhsT=xTs[:, :p], rhs=w1t[:], start=True, stop=True)
        hs = work.tile([128, 128], TDT, tag="hs")
        nc.vector.tensor_copy(out=hs[:p, :], in_=hp[:p, :128])
        nc.sync.dma_start(h_local[t0:t0 + p, :], hs[:p, :])

    if _os0.environ.get("GNN_STAGE") == "ph":
        return
    PIECE = cfg.PIECE
    if cfg.PIECES:
        for q in range(NSEG):
            nc.gpsimd.collective_compute(
                "AllGather", Alu.bypass,
                replica_groups=[list(range(cfg.CORES))],
                ins=[h_local[q * PIECE:(q + 1) * PIECE, :]],
                outs=[h1_full[q * SEG:(q + 1) * SEG, :]])
    else:
        nc.gpsimd.collective_compute(
            "AllGather", Alu.bypass,
            replica_groups=[list(range(cfg.CORES))],
            ins=[h_local[:, :]], outs=[h1_full[:, :]])

    # ---- per-layer ----
    zres = zpool.tile([128, NWPAD], f32, tag="z")
    stats1 = spool.tile([128, NW], f32, tag="s1")
    stats2 = spool.tile([128, NW], f32, tag="s2")

    def edge_layer(lyr, table):
        """Aggregate msgs into zres (channel-major, + bias); fill stats."""
        b_col = cols[f"b{lyr}"]
        win_seen = {}
        for sb in range(NSB):
            w0 = sb * SBW
            w1_ = min(w0 + SBW, NW)
            wt = {w: psum_agg.tile([128, 128], f32, tag="aggw", name=f"aggw{w}")
                  for w in range(w0, w1_)}
            for sg in range(NSEG):
                nch = sched.n_chunks_call[(sb, sg)]
                if nch == 0:
                    continue
                nsv = sched.n_sv_call[(sb, sg)]
                off = sched.chunk_off[(sb, sg)]
                soff = sched.sv_off[(sb, sg)]
                it = idxp.tile([128, 8 * CHMAX], i16, tag="it")
                nc.sync.dma_start(it[:, :8 * nch],
                                  meta[:, off * 8:(off + nch) * 8])
                sv = svp.tile([128, 2 * SVMAX], f32, tag="sv")
                nc.sync.dma_start(sv[:, :2 * nsv],
                                  slotval[:, soff * 2:(soff + nsv) * 2])
                msg = msgp.tile([128, CHMAX * 128], TDT, tag="msg")
                mview = msg[:, :nch * 128].rearrange("p (c e) -> p c e", e=128)
                gi = nc.gpsimd.dma_gather(
                    out_ap=mview, in_ap=table[sg * SEG:(sg + 1) * SEG, :],
                    idxs_ap=it[:, :8 * nch], num_idxs=nch * 128,
                    num_idxs_reg=nidx_reg(nch * 128), elem_size=128,
                    single_packet=False)
                _add_dep_helper(gi.ins, lib_inst.ins, sync=False,
                                reason="gpsimd library order")
                for (w, ch, poff, cap, svl) in sched.ops[sb][sg]:
                    oh = ohp.tile([128, 128], TDT, tag="oh")
                    nc.vector.tensor_scalar(
                        out=oh[:], in0=iota_row_h[:],
                        scalar1=sv[:, 2 * svl:2 * svl + 1],
                        scalar2=sv[:, 2 * svl + 1:2 * svl + 2],
                        op0=Alu.is_equal, op1=Alu.mult)
                    seen = win_seen.get(w, 0)
                    nc.tensor.matmul(
                        wt[w][:], lhsT=msg[:, ch * 128:(ch + 1) * 128],
                        rhs=oh[:], start=(seen == 0),
                        stop=(seen == sched.win_total[w] - 1))
                    win_seen[w] = seen + 1
            # drain superblock
            for w in range(w0, w1_):
                wdst = min(WIN, NPC - w * WIN)
                zsl = zres[:, w * 128:w * 128 + 128]
                nc.vector.tensor_scalar(
                    out=zsl, in0=wt[w][:], scalar1=b_col[:, 0:1], scalar2=None,
                    op0=Alu.add)
                nc.vector.tensor_reduce(
                    out=stats1[:, w:w + 1], in_=zres[:, w * 128:w * 128 + wdst],
                    axis=mybir.AxisListType.X, op=Alu.add)
                sq = scr.tile([128, 128], f32, tag="sq")
                nc.scalar.activation(
                    out=sq[:, :wdst], in_=zres[:, w * 128:w * 128 + wdst],
                    func=Act.Square, accum_out=stats2[:, w:w + 1])
        win_seen.clear()

    def bn_reduce(lyr):
        """AllReduce stats; returns (a_col, bb_col) affine tiles."""
        s_all = scr.tile([128, 2], f32, tag="sall")
        nc.vector.tensor_reduce(out=s_all[:, 0:1], in_=stats1[:, :NW],
                                axis=mybir.AxisListType.X, op=Alu.add)
        nc.vector.tensor_reduce(out=s_all[:, 1:2], in_=stats2[:, :NW],
                                axis=mybir.AxisListType.X, op=Alu.add)
        nc.sync.dma_start(bn_in[:, :], s_all[:, :])
        nc.gpsimd.collective_compute(
            "AllReduce", Alu.add,
            replica_groups=[list(range(cfg.CORES))],
            ins=[bn_in[:, :]], outs=[bn_out[lyr - 1, :, :]])
        st = colp.tile([128, 2], f32, tag=f"bnst{lyr}")
        nc.sync.dma_start(st[:, :], bn_out[lyr - 1, :, :])
        mu = colp.tile([128, 1], f32, tag=f"mu{lyr}")
        nc.vector.tensor_scalar(out=mu[:], in0=st[:, 0:1], scalar1=1.0 / N,
                                scalar2=None, op0=Alu.mult)
        e2 = colp.tile([128, 1], f32, tag=f"e2{lyr}")
        nc.vector.tensor_scalar(out=e2[:], in0=st[:, 1:2], scalar1=1.0 / N,
                                scalar2=None, op0=Alu.mult)
        var = colp.tile([128, 1], f32, tag=f"var{lyr}")
        nc.vector.tensor_tensor(out=var[:], in0=mu[:], in1=mu[:], op=Alu.mult)
        nc.vector.tensor_tensor(out=var[:], in0=e2[:], in1=var[:],
                                op=Alu.subtract)
        nc.vector.tensor_scalar(out=var[:], in0=var[:], scalar1=cfg.BN_EPS,
                                scalar2=None, op0=Alu.add)
        inv = colp.tile([128, 1], f32, tag=f"inv{lyr}")
        nc.vector.reciprocal(out=inv[:], in_=var[:])
        rstd = colp.tile([128, 1], f32, tag=f"rstd{lyr}")
        nc.scalar.sqrt(out=rstd[:], in_=inv[:])
        a = colp.tile([128, 1], f32, tag=f"a{lyr}")
        nc.vector.tensor_tensor(out=a[:], in0=cols[f"g{lyr}"][:], in1=rstd[:],
                                op=Alu.mult)
        bb = colp.tile([128, 1], f32, tag=f"bb{lyr}")
        nc.vector.tensor_tensor(out=bb[:], in0=mu[:], in1=a[:], op=Alu.mult)
        nc.vector.tensor_tensor(out=bb[:], in0=cols[f"be{lyr}"][:], in1=bb[:],
                                op=Alu.subtract)
        return a, bb

    import os as _os
    _stage = _os.environ.get("GNN_STAGE", "full")
    _reps = int(_os.environ.get("GNN_REPS", "1"))
    _comp = _os.environ.get("GNN_COMP", "")
    if _comp:
        # amplification experiment: repeat one component _reps times
        def gather_variant(table, mode):
            """mode: none|plain|tr|big|sb — idx DMA always included."""
            if mode == "sb":
                TPR = 128
                tbl = const.tile([128, TPR * 128], TDT, tag="sbtable")
                nc.sync.dma_start(tbl[:, :], table[0:TPR * 128, :]
                                  .rearrange("(p t) e -> p (t e)", p=128))
            for sb in range(NSB):
                for sg in range(NSEG):
                    nch = sched.n_chunks_call[(sb, sg)]
                    if nch == 0:
                        continue
                    off = sched.chunk_off[(sb, sg)]
                    it = idxp.tile([128, 8 * CHMAX], i16, tag="it")
                    nc.sync.dma_start(it[:, :8 * nch],
                                      meta[:, off * 8:(off + nch) * 8])
                    if mode == "none":
                        continue
                    msg = msgp.tile([128, CHMAX * 128], TDT, tag="msg")
                    if mode == "plain":
                        mview = msg[:, :nch * 128].rearrange(
                            "p (c e) -> p c e", e=128)
                        gi = nc.gpsimd.dma_gather(
                            out_ap=mview,
                            in_ap=table[sg * SEG:(sg + 1) * SEG, :],
                            idxs_ap=it[:, :8 * nch], num_idxs=nch * 128,
                            num_idxs_reg=nidx_reg(nch * 128), elem_size=128,
                            single_packet=False)
                    elif mode == "sp":
                        mview = msg[:, :nch * 128].rearrange(
                            "p (c e) -> p c e", e=128)
                        gi = nc.gpsimd.dma_gather(
                            out_ap=mview,
                            in_ap=table[sg * SEG:(sg + 1) * SEG, :],
                            idxs_ap=it[:, :8 * nch], num_idxs=nch * 128,
                            num_idxs_reg=nidx_reg(nch * 128), elem_size=128,
                            single_packet=True)
                    elif mode == "big":
                        mview = msg[:, :nch * 128].rearrange(
                            "p (c e) -> p c e", e=256)
                        gi = nc.gpsimd.dma_gather(
                            out_ap=mview,
                            in_ap=table[sg * SEG:(sg + 1) * SEG, :]
                            .rearrange("(a b) e -> a (b e)", b=2),
                            idxs_ap=it[:, :4 * nch], num_idxs=nch * 64,
                            num_idxs_reg=nidx_reg(nch * 64), elem_size=256,
                            single_packet=False)
                    elif mode == "tr":
                        mview = msg[:, :nch * 128].rearrange(
                            "p (c e) -> p c e", c=1)
                        gi = nc.gpsimd.dma_gather(
                            out_ap=mview,
                            in_ap=table[sg * SEG:(sg + 1) * SEG, :],
                            idxs_ap=it[:, :8 * nch], num_idxs=nch * 128,
                            num_idxs_reg=nidx_reg(nch * 128), elem_size=128,
                            transpose=True, single_packet=False)
                    elif mode == "sb":
                        TPR = 128
                        itm = idxp.tile([128, 8 * CHMAX], i16, tag="itm")
                        nc.vector.tensor_scalar(
                            out=itm[:, :8 * nch], in0=it[:, :8 * nch],
                            scalar1=16383, scalar2=None,
                            op0=Alu.bitwise_and)
                        mview = msg[:, :nch * 128].rearrange(
                            "p (c e) -> p c e", c=1)
                        gi = nc.gpsimd.dma_gather(
                            out_ap=mview,
                            in_ap=tbl[:, :],
                            idxs_ap=itm[:, :8 * nch], num_idxs=nch * 128,
                            num_idxs_reg=nidx_reg(nch * 128), elem_size=128,
                            transpose=True, single_packet=False,
                            sbuf_tokens_per_rank=TPR,
                            sbuf_free_dim_per_rank=TPR * 256)
                    _add_dep_helper(gi.ins, lib_inst.ins, sync=False,
                                    reason="gpsimd library order")

        def gather_only(table):
            for sb in range(NSB):
                for sg in range(NSEG):
                    nch = sched.n_chunks_call[(sb, sg)]
                    if nch == 0:
                        continue
                    off = sched.chunk_off[(sb, sg)]
                    it = idxp.tile([128, 8 * CHMAX], i16, tag="it")
                    nc.sync.dma_start(it[:, :8 * nch],
                                      meta[:, off * 8:(off + nch) * 8])
                    msg = msgp.tile([128, CHMAX * 128], TDT, tag="msg")
                    mview = msg[:, :nch * 128].rearrange("p (c e) -> p c e", e=128)
                    gi = nc.gpsimd.dma_gather(
                        out_ap=mview, in_ap=table[sg * SEG:(sg + 1) * SEG, :],
                        idxs_ap=it[:, :8 * nch], num_idxs=nch * 128,
                        num_idxs_reg=nidx_reg(nch * 128), elem_size=128,
                        single_packet=False)
                    _add_dep_helper(gi.ins, lib_inst.ins, sync=False,
                                    reason="gpsimd library order")

        def onehot_only():
            for sb in range(NSB):
                for sg in range(NSEG):
                    nsv = sched.n_sv_call[(sb, sg)]
                    if nsv == 0:
                        continue
                    soff = sched.sv_off[(sb, sg)]
                    sv = svp.tile([128, 2 * SVMAX], f32, tag="sv")
                    nc.sync.dma_start(sv[:, :2 * nsv],
                                      slotval[:, soff * 2:(soff + nsv) * 2])
                    for ci in range(nsv):
                        oh = ohp.tile([128, 128], TDT, tag="oh")
                        nc.vector.tensor_scalar(
                            out=oh[:], in0=iota_row_h[:],
                            scalar1=sv[:, 2 * ci:2 * ci + 1],
                            scalar2=sv[:, 2 * ci + 1:2 * ci + 2],
                            op0=Alu.is_equal, op1=Alu.mult)

        for _r in range(_reps):
            if _comp == "gather":
                gather_only(h1_full)
            elif _comp.startswith("gath"):
                gather_variant(h1_full, _comp[4:])
            elif _comp == "onehot":
                onehot_only()
            elif _comp == "edge":
                edge_layer(1, h1_full)
            elif _comp == "ag":
                for q in range(NSEG):
                    nc.gpsimd.collective_compute(
                        "AllGather", Alu.bypass,
                        replica_groups=[list(range(cfg.CORES))],
                        ins=[h_local[q * PIECE:(q + 1) * PIECE, :]],
                        outs=[h1_full[q * SEG:(q + 1) * SEG, :]])
        return
    if _stage == "a":
        return
    # ======== layer 1 ========
    edge_layer(1, h1_full)
    if _stage == "l1":
        return
    a1, bb1 = bn_reduce(1)
    if _stage == "bn1":
        return
    for w in range(NW):
        zsl = zres[:, w * 128:(w + 1) * 128]
        nc.scalar.activation(out=zsl, in_=zsl, func=Act.Relu,
                             scale=a1[:, 0:1], bias=bb1[:, 0:1])

    # h2_local = z1 @ W2 (z1 channel-major resident) -> node-major DRAM
    for c0 in range(0, NWPAD, 512):
        cw = min(512, NWPAD - c0)
        hp = psum_misc.tile([128, 512], f32, tag="pm")
        nc.tensor.matmul(hp[:, :cw], lhsT=w2t[:], rhs=zres[:, c0:c0 + cw],
                         start=True, stop=True)
        hsb = work.tile([128, 512], f32, tag="h2s")
        nc.vector.tensor_copy(out=hsb[:, :cw], in_=hp[:, :cw])
        for j0 in range(0, cw, 128):
            n0 = c0 + j0
            cnt = min(128, NPC - n0)
            if cnt <= 0:
                break
            tp = psum_misc.tile([128, 512], f32, tag="pm")
            nc.tensor.transpose(tp[:, :128], hsb[:, j0:j0 + 128], ident[:])
            ts_ = work.tile([128, 128], TDT, tag="tnmh")
            nc.vector.tensor_copy(out=ts_[:cnt, :], in_=tp[:cnt, :128])
            nc.sync.dma_start(h2_local[n0:n0 + cnt, :], ts_[:cnt, :])

    if cfg.PIECES:
        for q in range(NSEG):
            nc.gpsimd.collective_compute(
                "AllGather", Alu.bypass,
                replica_groups=[list(range(cfg.CORES))],
                ins=[h2_local[q * PIECE:(q + 1) * PIECE, :]],
                outs=[h2_full[q * SEG:(q + 1) * SEG, :]])
    else:
        nc.gpsimd.collective_compute(
            "AllGather", Alu.bypass,
            replica_groups=[list(range(cfg.CORES))],
            ins=[h2_local[:, :]], outs=[h2_full[:, :]])

    if _stage == "h2":
        return
    # ======== layer 2 ========
    edge_layer(2, h2_full)
    a2, bb2 = bn_reduce(2)
    for w in range(NW):
        wdst = min(WIN, NPC - w * WIN)
        ocm = work.tile([128, 128], f32, tag="ocm")
        nc.vector.tensor_scalar(
            out=ocm[:], in0=zres[:, w * 128:(w + 1) * 128],
            scalar1=a2[:, 0:1], scalar2=bb2[:, 0:1],
            op0=Alu.mult, op1=Alu.add)
        tp = psum_misc.tile([128, 512], f32, tag="pm")
        nc.tensor.transpose(tp[:, :128], ocm[:, :], ident[:])
        ts_ = work.tile([128, 128], f32, tag="tnm")
        nc.vector.tensor_copy(out=ts_[:wdst, :], in_=tp[:wdst, :128])
        nc.sync.dma_start(y[w * 128:w * 128 + wdst, :], ts_[:wdst, :])


def build_program(cfg: Cfg, sched: Sched):
    """Create Bass program; returns (nc, input names)."""
    import concourse.bacc as bacc
    import concourse.mybir as mybir
    from concourse.tile import TileContext
    _apply_tile_patch()

    f32 = mybir.dt.float32
    nc = bacc.Bacc(num_devices=cfg.CORES)
    CT = sched.CT
    import os as _os
    if _os.environ.get("GNN_TINY"):
        # shrink all big external tensors to probe host<->device transfer cost
        tensors = {
            "x_shard": nc.dram_tensor("x_shard", [128, cfg.D], f32,
                                      kind="ExternalInput"),
            "consts": nc.dram_tensor("consts", [128, 129], f32,
                                     kind="ExternalInput"),
            "meta": nc.dram_tensor("meta", [128, 8], mybir.dt.int16,
                                   kind="ExternalInput"),
            "slotval": nc.dram_tensor("slotval", [128, 2], f32,
                                      kind="ExternalInput"),
            "W1": nc.dram_tensor("W1", [128, 128], f32, kind="ExternalInput"),
            "W2": nc.dram_tensor("W2", [128, 128], f32, kind="ExternalInput"),
            "b1": nc.dram_tensor("b1", [128, 1], f32, kind="ExternalInput"),
            "g1": nc.dram_tensor("g1", [128, 1], f32, kind="ExternalInput"),
            "be1": nc.dram_tensor("be1", [128, 1], f32, kind="ExternalInput"),
            "b2": nc.dram_tensor("b2", [128, 1], f32, kind="ExternalInput"),
            "g2": nc.dram_tensor("g2", [128, 1], f32, kind="ExternalInput"),
            "be2": nc.dram_tensor("be2", [128, 1], f32, kind="ExternalInput"),
            "y": nc.dram_tensor("y", [128, cfg.D], f32, kind="ExternalOutput"),
        }
        with TileContext(nc) as tc:
            import concourse.mybir as _mb
            with tc.tile_pool(name="tiny", bufs=1) as t:
                tt = t.tile([128, 129], f32)
                nc.sync.dma_start(tt[:], tensors["consts"][:, :])
                ty = t.tile([128, cfg.D], f32)
                nc.vector.tensor_scalar(out=ty[:], in0=tt[:, :128], scalar1=2.0,
                                        scalar2=None, op0=_mb.AluOpType.mult)
                nc.sync.dma_start(tensors["y"][:, :], ty[:])
        if not nc.is_finalized():
            nc.finalize()
        return nc
    tensors = {
        "x_shard": nc.dram_tensor("x_shard", [cfg.NPC, cfg.D], f32,
                                  kind="ExternalInput"),
        "consts": nc.dram_tensor("consts", [128, 129], f32,
                                 kind="ExternalInput"),
        "meta": nc.dram_tensor("meta", [128, 8 * CT], mybir.dt.int16,
                               kind="ExternalInput"),
        "slotval": nc.dram_tensor("slotval", [128, 2 * sched.SVC], f32,
                                  kind="ExternalInput"),
        "W1": nc.dram_tensor("W1", [128, 128], f32, kind="ExternalInput"),
        "W2": nc.dram_tensor("W2", [128, 128], f32, kind="ExternalInput"),
        "b1": nc.dram_tensor("b1", [128, 1], f32, kind="ExternalInput"),
        "g1": nc.dram_tensor("g1", [128, 1], f32, kind="ExternalInput"),
        "be1": nc.dram_tensor("be1", [128, 1], f32, kind="ExternalInput"),
        "b2": nc.dram_tensor("b2", [128, 1], f32, kind="ExternalInput"),
        "g2": nc.dram_tensor("g2", [128, 1], f32, kind="ExternalInput"),
        "be2": nc.dram_tensor("be2", [128, 1], f32, kind="ExternalInput"),
        "y": nc.dram_tensor("y", [cfg.NPC, cfg.D], f32, kind="ExternalOutput"),
    }
    with TileContext(nc) as tc:
        build(nc, tc, cfg, sched, tensors)
        tc._gnn_exitstack.close()
    if not nc.is_finalized():
        nc.finalize()
    return nc


def make_consts():
    c = np.zeros((128, 129), np.float32)
    c[:, :128] = np.arange(128, dtype=np.float32)[None, :]
    c[:, 128] = np.arange(128, dtype=np.float32)
    return c


def kernel_run(inputs: dict, cfg: Cfg):
    """Full flow: prep -> build -> run on 8 cores -> assemble output."""
    import numpy as np
    from concourse.bass_utils import run_bass_kernel_spmd

    x = np.asarray(inputs["x"], np.float32)
    ei = np.asarray(inputs["edge_index"])
    ew = np.asarray(inputs["edge_weight"], np.float32)
    sched, per_core = prep(ei, ew, cfg)
    nc = build_program(cfg, sched)

    com = {
        "W1": np.ascontiguousarray(inputs["W1"], dtype=np.float32),
        "W2": np.ascontiguousarray(inputs["W2"], dtype=np.float32),
        "consts": make_consts(),
    }
    for nm in ("b1", "g1", "be1", "b2", "g2", "be2"):
        com[nm] = np.ascontiguousarray(
            np.asarray(inputs[nm], np.float32).reshape(128, 1))
    in_maps = []
    for ci in range(cfg.CORES):
        m = dict(com)
        m["x_shard"] = np.ascontiguousarray(x[ci * cfg.NPC:(ci + 1) * cfg.NPC])
        m["meta"] = per_core[ci]["meta"]
        m["slotval"] = per_core[ci]["slotval"]
        in_maps.append(m)
    res = run_bass_kernel_spmd(nc, in_maps, core_ids=list(range(cfg.CORES)))
    out = np.concatenate([r["y"] for r in res.results], axis=0)
    return out, res


def kernel(**inputs) -> np.ndarray:
    cfg = Cfg()
    out, _ = kernel_run(inputs, cfg)
    return out



# revision 24
# speedup vs baseline: 1.8811x; 1.8811x over previous
"""Trainium2 Bass kernel for a 2-layer GCN (GCNConv+BN+ReLU, GCNConv+BN).

Self-contained: kernel(**inputs) takes the full unsharded inputs, shards
destinations across 8 NeuronCores (edges partitioned by destination, one-hot
matmul scatter-add on the TensorEngine, dma_gather message fetch from fp16
feature tables, piece-wise AllGather for tables, AllReduce for BN stats),
and returns the full [100000, 128] float32 output.
"""
import numpy as np


def _apply_tile_patch():
    """This walrus build allows only 1 sync wait per CTRL instruction and
    rejects long semaphore range clears; spread Tile's exit-drain waits
    across nops and chunk the sem recycles."""
    import concourse.tile as tile_mod
    import concourse.mybir as mybir
    from concourse.vector_clock import ScopedClock

    MAXW = 1

    def _patched_drain_and_barrier(self, tick_clock, wait_clock):
        nc = self.nc
        tmp = nc.sync.nop(nofuse=True, hint="drain_wait_spread")
        wait_clock.add_sem_waits(tmp.ins, ScopedClock({None: tick_clock.global_clock}))
        waits = list(tmp.ins.sync_info.on_wait or [])
        if len(waits) > MAXW:
            tmp.ins.sync_info = mybir.SyncInfo(on_wait=waits[:MAXW], on_update=[])
            for i in range(MAXW, len(waits), MAXW):
                extra = nc.sync.nop(nofuse=True, hint="drain_wait_spread")
                extra.ins.sync_info = mybir.SyncInfo(
                    on_wait=waits[i:i + MAXW], on_update=[])
        nc.sync.drain()
        nc.all_engine_barrier()
        assert self.sems is not None
        popped = nc._tile_sem_poison_stack.pop()
        assert popped is self._sem_poison
        sems = list(self.sems.allocated().values())
        for i in range(0, len(sems), 8):
            nc.clear_and_free_semaphores(sems[i:i + 8])
        nc.all_engine_barrier()

    tile_mod.TileContext._drain_and_barrier = _patched_drain_and_barrier






from dataclasses import dataclass, field


@dataclass
class Cfg:
    N: int = 100000
    D: int = 128
    CORES: int = 8
    WIN: int = 128
    SBW: int = 6          # windows per superblock (PSUM-resident agg tiles)
    SEG: int = 25000      # gather table segment rows (int16 index reach)
    BN_EPS: float = 1e-5
    FP16: bool = True     # fp16 feature tables / messages / one-hots
    PIECES: bool = True   # piece-wise pipelined AllGather

    @property
    def PIECE(self):
        # per-core rows contributed to one pipelined AllGather piece
        return self.SEG // self.CORES

    @property
    def NPC(self):
        return self.N // self.CORES

    @property
    def NW(self):
        return (self.NPC + self.WIN - 1) // self.WIN

    @property
    def NSB(self):
        return (self.NW + self.SBW - 1) // self.SBW

    @property
    def NSEG(self):
        return (self.N + self.SEG - 1) // self.SEG


@dataclass
class Sched:
    # ops[sb][seg] = list of (w, chunk_local, part_off, cap, sv_col_local):
    #   one scatter-matmul per op. Full chunks have part_off=0, cap=128.
    #   Remainder chunks pack several windows' tails; each window-use is an
    #   op with its own slotval column (foreign partitions have val=0).
    ops: list = field(default_factory=list)
    # full[sb][seg][w] = (first chunk col local, n full chunks)
    layout: list = field(default_factory=list)
    chunk_off: dict = field(default_factory=dict)   # (sb, seg) -> first chunk col
    sv_off: dict = field(default_factory=dict)      # (sb, seg) -> first sv col
    n_chunks_call: dict = field(default_factory=dict)  # (sb, seg) -> chunks in call
    n_sv_call: dict = field(default_factory=dict)   # (sb, seg) -> sv cols in call
    CT: int = 0                                     # total chunk columns
    SVC: int = 0                                    # total slotval columns
    win_total: dict = field(default_factory=dict)   # w -> total ops (all segs)


def make_schedule(counts_max, cfg: Cfg) -> Sched:
    """counts_max[w, s] = max edge count over cores for (window w, segment s)."""
    s = Sched()
    ct = 0
    svc = 0
    for sb in range(cfg.NSB):
        w0, w1 = sb * cfg.SBW, min((sb + 1) * cfg.SBW, cfg.NW)
        per_seg_ops = []
        per_seg_layout = []
        for seg in range(cfg.NSEG):
            ops = []
            lay = {}
            nch = 0
            # full chunks per window
            rems = []
            for w in range(w0, w1):
                c = int(counts_max[w, seg])
                full = c // 128
                if full:
                    lay[w] = (nch, full)
                    for k in range(full):
                        ops.append((w, nch + k, 0, 128, None))
                    nch += full
                if c % 128:
                    rems.append((w, c % 128))
            # first-fit decreasing pack of remainders into shared chunks
            rems.sort(key=lambda t: -t[1])
            bins = []  # (used, chunk_idx, list of (w, off, cnt))
            for w, r in rems:
                placed = False
                for b in bins:
                    if b[0] + r <= 128:
                        b[2].append((w, b[0], r))
                        b[0] += r
                        placed = True
                        break
                if not placed:
                    bins.append([r, nch + len(bins), [(w, 0, r)]])
            for used, ci, uses in bins:
                for (w, off, r) in uses:
                    ops.append((w, ci, off, r, None))
            nch += len(bins)
            # assign sv columns in op order
            ops2 = []
            for (w, ci, off, cap, _) in ops:
                ops2.append((w, ci, off, cap, len(ops2)))
                s.win_total[w] = s.win_total.get(w, 0) + 1
            s.chunk_off[(sb, seg)] = ct
            s.sv_off[(sb, seg)] = svc
            s.n_chunks_call[(sb, seg)] = nch
            s.n_sv_call[(sb, seg)] = len(ops2)
            ct += nch
            svc += len(ops2)
            per_seg_ops.append(ops2)
            per_seg_layout.append(lay)
        s.ops.append(per_seg_ops)
        s.layout.append(per_seg_layout)
    s.CT = ct
    s.SVC = svc
    return s


def prep(edge_index: np.ndarray, edge_weight: np.ndarray, cfg: Cfg):
    """Host preprocessing: normalization, dest-sharding, chunk packing.

    Returns (sched, per_core) where per_core[i] = dict with idx16 [128, 8*CT]
    int16 and slotval [128, 2*CT] f32.
    """
    N = cfg.N
    row = np.concatenate([edge_index[0], np.arange(N, dtype=np.int64)]).astype(np.int64)
    col = np.concatenate([edge_index[1], np.arange(N, dtype=np.int64)]).astype(np.int64)
    w = np.concatenate([edge_weight.astype(np.float64), np.ones(N)])

    deg = np.bincount(col, weights=w, minlength=N)
    dinv = np.where(deg > 0, 1.0 / np.sqrt(np.maximum(deg, 1e-12)), 0.0)
    norm = (dinv[row] * w * dinv[col]).astype(np.float32)

    core = col // cfg.NPC
    d = col % cfg.NPC
    win = d // cfg.WIN
    slot = d % cfg.WIN
    if cfg.PIECES:
        # table row of node n (piece-wise AllGather layout):
        #   c = n // NPC, r = n % NPC, q = r // PIECE
        #   trow = q * SEG + c * PIECE + r % PIECE ; seg = q
        src_c = row // cfg.NPC
        src_r = row % cfg.NPC
        seg = src_r // cfg.PIECE
        srcid = (src_c * cfg.PIECE + src_r % cfg.PIECE).astype(np.int16)
    else:
        seg = row // cfg.SEG
        srcid = (row % cfg.SEG).astype(np.int16)

    # group id per edge: (core, sb, seg, win)
    sb = win // cfg.SBW
    NW, NSEG = cfg.NW, cfg.NSEG
    gid = ((core * cfg.NSB + sb) * NSEG + seg) * NW + win
    order = np.argsort(gid, kind="stable")
    gid_s = gid[order]
    srcid_s = srcid[order]
    slot_s = slot[order].astype(np.float32)
    norm_s = norm[order]

    # counts per (core, win, seg)
    n_gids = cfg.CORES * cfg.NSB * NSEG * NW
    cnt = np.bincount(gid, minlength=n_gids)
    cntr = cnt.reshape(cfg.CORES, cfg.NSB, NSEG, NW)
    w_idx = np.arange(NW)
    # counts[c, w, s]: only the (sb = w // SBW) plane is populated
    counts = cntr[:, w_idx // cfg.SBW, :, w_idx]      # [NW, CORES, NSEG]
    counts = counts.transpose(1, 0, 2)                # [CORES, NW, NSEG]
    counts_max = counts.max(axis=0)  # [NW, NSEG]

    sched = make_schedule(counts_max, cfg)
    CT = sched.CT
    SVC = sched.SVC

    starts = np.zeros(n_gids + 1, dtype=np.int64)
    np.cumsum(cnt, out=starts[1:])

    per_core = []
    for ci in range(cfg.CORES):
        src_p = np.zeros((CT, 128), dtype=np.int16)
        slot_p = np.zeros((SVC, 128), dtype=np.float32)
        val_p = np.zeros((SVC, 128), dtype=np.float32)
        for sbi in range(cfg.NSB):
            for sg in range(NSEG):
                cbase = sched.chunk_off[(sbi, sg)]
                svbase = sched.sv_off[(sbi, sg)]
                for (wg, ch, off, cap, svl) in sched.ops[sbi][sg]:
                    g = ((ci * cfg.NSB + sbi) * NSEG + sg) * NW + wg
                    a, b = starts[g], starts[g + 1]
                    n = b - a
                    # op order per window follows chunk order: figure out
                    # which slice of this core's edges belongs to this op
                    lay = sched.layout[sbi][sg].get(wg)
                    if lay is not None:
                        c0, nfull = lay
                        if ch < c0 + nfull:
                            # full chunk k of window wg
                            k = ch - c0
                            a2 = a + min(n, k * 128)
                            b2 = a + min(n, (k + 1) * 128)
                        else:
                            a2 = a + min(n, nfull * 128)
                            b2 = b
                    else:
                        a2, b2 = a, b
                    m = b2 - a2
                    assert m <= cap
                    if m <= 0:
                        continue
                    fl = slice((cbase + ch) * 128 + off,
                               (cbase + ch) * 128 + off + m)
                    src_p.reshape(-1)[fl] = srcid_s[a2:b2]
                    svfl = slice((svbase + svl) * 128 + off,
                                 (svbase + svl) * 128 + off + m)
                    slot_p.reshape(-1)[svfl] = slot_s[a2:b2]
                    val_p.reshape(-1)[svfl] = norm_s[a2:b2]
        # meta [128, 8*CT] int16: per (sb,seg) call gather idxs at col 8*off,
        # flat idx j -> [j % 16, j // 16]
        meta = np.zeros((128, 8 * CT), dtype=np.int16)
        for sbi in range(cfg.NSB):
            for sg in range(NSEG):
                nch = sched.n_chunks_call[(sbi, sg)]
                if not nch:
                    continue
                o = sched.chunk_off[(sbi, sg)]
                flat = src_p.reshape(-1)[o * 128:(o + nch) * 128]
                wrapped = flat.reshape(-1, 16).T  # [16, nch*8]
                meta[:, o * 8:(o + nch) * 8] = np.tile(wrapped, (8, 1))
        sv = np.stack([slot_p, val_p], axis=-1)  # [SVC, 128, 2]
        slotval = np.ascontiguousarray(
            sv.transpose(1, 0, 2).reshape(128, 2 * SVC))
        per_core.append({"meta": meta, "slotval": slotval})
    return sched, per_core


def build(nc, tc, cfg: Cfg, sched: Sched, tensors):
    """Emit the kernel into TileContext tc. tensors: dict of dram handles."""
    import concourse.mybir as mybir
    from concourse.bass import ts as _ts  # noqa: F401

    f32 = mybir.dt.float32
    f16 = mybir.dt.float16
    TDT = f16 if cfg.FP16 else f32
    i16 = mybir.dt.int16
    i32 = mybir.dt.int32
    Alu = mybir.AluOpType
    Act = mybir.ActivationFunctionType

    N, D, NPC, WIN, NW, NSB, SBW = (
        cfg.N, cfg.D, cfg.NPC, cfg.WIN, cfg.NW, cfg.NSB, cfg.SBW)
    NSEG, SEG = cfg.NSEG, cfg.SEG
    NWPAD = NW * WIN

    x = tensors["x_shard"]
    meta = tensors["meta"]
    slotval = tensors["slotval"]
    W1, W2 = tensors["W1"], tensors["W2"]
    bias = {1: tensors["b1"], 2: tensors["b2"]}
    gam = {1: tensors["g1"], 2: tensors["g2"]}
    bet = {1: tensors["be1"], 2: tensors["be2"]}
    y = tensors["y"]

    CHMAX = max(sched.n_chunks_call.values())
    SVMAX = max(sched.n_sv_call.values())

    from contextlib import ExitStack
    es = tc._gnn_exitstack = ExitStack()
    const = es.enter_context(tc.tile_pool(name="const", bufs=1))
    zpool = es.enter_context(tc.tile_pool(name="zres", bufs=1))
    spool = es.enter_context(tc.tile_pool(name="stats", bufs=1))
    work = es.enter_context(tc.tile_pool(name="work", bufs=3))
    ohp = es.enter_context(tc.tile_pool(name="oh", bufs=12))
    msgp = es.enter_context(tc.tile_pool(name="msg", bufs=3))
    idxp = es.enter_context(tc.tile_pool(name="idx", bufs=3))
    svp = es.enter_context(tc.tile_pool(name="sv", bufs=3))
    scr = es.enter_context(tc.tile_pool(name="scr", bufs=2))
    colp = es.enter_context(tc.tile_pool(name="col", bufs=1))
    psum_agg = es.enter_context(tc.tile_pool(name="pagg", bufs=SBW, space="PSUM"))
    psum_misc = es.enter_context(tc.tile_pool(name="pmisc", bufs=2, space="PSUM"))
    dram = es.enter_context(tc.tile_pool(name="dram", bufs=1, space="DRAM"))

    # ---- constants (iota rows/col provided by host via "consts" input) ----
    from concourse import library_config
    from concourse.bass import _add_dep_helper
    lib_inst = nc.gpsimd.load_library(library_config.mlp)
    _nreg_cache = {}

    def nidx_reg(v):
        if v not in _nreg_cache:
            r = nc.gpsimd.alloc_register(f"nidx_{v}")
            nc.gpsimd.reg_mov(r, v)
            _nreg_cache[v] = r
        return _nreg_cache[v]
    consts_t = const.tile([128, 129], f32, name="consts_t")
    nc.sync.dma_start(consts_t[:], tensors["consts"][:, :])
    iota_row = consts_t[:, 0:128]
    iota_col = consts_t[:, 128:129]
    ident = const.tile([128, 128], f32)
    nc.vector.tensor_scalar(
        out=ident[:], in0=iota_row, scalar1=iota_col, scalar2=None,
        op0=Alu.is_equal)
    iota_row_h = const.tile([128, 128], f16)
    nc.vector.tensor_copy(out=iota_row_h[:], in_=iota_row)

    # per-channel params as [128,1] columns
    cols = {}
    for nm in ("b1", "g1", "be1", "b2", "g2", "be2"):
        t = colp.tile([128, 1], f32, tag=nm)
        nc.sync.dma_start(t[:], tensors[nm][:, :])
        cols[nm] = t

    # edge metadata resident in SBUF for both layers (one big DMA each)
    CT_ = sched.CT
    SVC_ = sched.SVC
    itall = const.tile([128, 8 * CT_], i16, tag="itall")
    nc.sync.dma_start(itall[:], meta[:, :])
    svall = const.tile([128, 2 * SVC_], f32, tag="svall")
    nc.sync.dma_start(svall[:], slotval[:, :])

    # ---- DRAM scratch ----
    h_local = dram.tile([NPC, D], TDT, tag="h_local")
    import os as _os_sh
    sh = _os_sh.environ.get("GNN_AG_SPACE", "Shared")
    h1_full = nc.dram_tensor("h1_full", [N, D], TDT, kind="Internal",
                             addr_space=sh)
    h2_full = nc.dram_tensor("h2_full", [N, D], TDT, kind="Internal",
                             addr_space=sh)
    h2_local = dram.tile([NPC, D], TDT, tag="h2_local")
    bn_in = dram.tile([128, 2], f32, tag="bn_in")
    bn_out = dram.tile([2, 128, 2], f32, tag="bn_out")

    import os as _os0
    if _os0.environ.get("GNN_STAGE") == "w":
        return
    # ---- phase A: h1_local = x @ W1; x arrives channel-major [D, NPC] ----
    w1t = const.tile([128, 128], f32, tag="w1")
    nc.sync.dma_start(w1t[:], W1[:, :])
    w2t = const.tile([128, 128], f32, tag="w2")
    nc.sync.dma_start(w2t[:], W2[:, :])

    XB = 8
    NWF = NPC // 128          # full 128-node windows
    TAIL = NPC % 128
    for g0 in range(0, NWF, XB):
        kcnt = min(XB, NWF - g0)
        xt = work.tile([128, XB * 128], f32, tag="xt")
        nc.sync.dma_start(xt[:, :kcnt * 128],
                          x[:, g0 * 128:(g0 + kcnt) * 128])
        hs = work.tile([128, XB * 128], TDT, tag="hs")
        for k in range(kcnt):
            hp = psum_misc.tile([128, 512], f32, tag="pm")
            nc.tensor.matmul(hp[:, :128], lhsT=xt[:, k * 128:(k + 1) * 128],
                             rhs=w1t[:], start=True, stop=True)
            nc.vector.tensor_copy(out=hs[:, k * 128:(k + 1) * 128],
                                  in_=hp[:, :128])
        nc.sync.dma_start(
            h_local[g0 * 128:(g0 + kcnt) * 128, :]
            .rearrange("(k p) c -> p k c", p=128),
            hs[:, :kcnt * 128].rearrange("p (k c) -> p k c", c=128))
    if TAIL:
        t0 = NWF * 128
        xt = work.tile([128, 128], f32, tag="xtt")
        nc.sync.dma_start(xt[:, :TAIL], x[:, t0:t0 + TAIL])
        hp = psum_misc.tile([128, 512], f32, tag="pm")
        nc.tensor.matmul(hp[:TAIL, :128], lhsT=xt[:, :TAIL], rhs=w1t[:],
                         start=True, stop=True)
        hs = work.tile([128, 128], TDT, tag="hst")
        nc.vector.tensor_copy(out=hs[:TAIL, :], in_=hp[:TAIL, :128])
        nc.sync.dma_start(h_local[t0:t0 + TAIL, :], hs[:TAIL, :])

    if _os0.environ.get("GNN_STAGE") == "ph":
        return
    PIECE = cfg.PIECE
    if cfg.PIECES:
        for q in range(NSEG):
            nc.gpsimd.collective_compute(
                "AllGather", Alu.bypass,
                replica_groups=[list(range(cfg.CORES))],
                ins=[h_local[q * PIECE:(q + 1) * PIECE, :]],
                outs=[h1_full[q * SEG:(q + 1) * SEG, :]])
    else:
        nc.gpsimd.collective_compute(
            "AllGather", Alu.bypass,
            replica_groups=[list(range(cfg.CORES))],
            ins=[h_local[:, :]], outs=[h1_full[:, :]])

    # ---- per-layer ----
    zres = zpool.tile([128, NWPAD], f32, tag="z")
    stats1 = spool.tile([128, NW], f32, tag="s1")
    stats2 = spool.tile([128, NW], f32, tag="s2")

    def edge_layer(lyr, table):
        """Aggregate msgs into zres (channel-major, + bias); fill stats."""
        b_col = cols[f"b{lyr}"]
        win_seen = {}
        for sb in range(NSB):
            w0 = sb * SBW
            w1_ = min(w0 + SBW, NW)
            wt = {w: psum_agg.tile([128, 128], f32, tag="aggw", name=f"aggw{w}")
                  for w in range(w0, w1_)}
            for sg in range(NSEG):
                nch = sched.n_chunks_call[(sb, sg)]
                if nch == 0:
                    continue
                off = sched.chunk_off[(sb, sg)]
                soff = sched.sv_off[(sb, sg)]
                msg = msgp.tile([128, CHMAX * 128], TDT, tag="msg")
                mview = msg[:, :nch * 128].rearrange("p (c e) -> p c e", e=128)
                gi = nc.gpsimd.dma_gather(
                    out_ap=mview, in_ap=table[sg * SEG:(sg + 1) * SEG, :],
                    idxs_ap=itall[:, off * 8:(off + nch) * 8],
                    num_idxs=nch * 128,
                    num_idxs_reg=nidx_reg(nch * 128), elem_size=128,
                    single_packet=False)
                _add_dep_helper(gi.ins, lib_inst.ins, sync=False,
                                reason="gpsimd library order")
                for (w, ch, poff, cap, svl) in sched.ops[sb][sg]:
                    sc = soff + svl
                    oh = ohp.tile([128, 128], TDT, tag="oh")
                    nc.vector.tensor_scalar(
                        out=oh[:], in0=iota_row_h[:],
                        scalar1=svall[:, 2 * sc:2 * sc + 1],
                        scalar2=svall[:, 2 * sc + 1:2 * sc + 2],
                        op0=Alu.is_equal, op1=Alu.mult)
                    seen = win_seen.get(w, 0)
                    nc.tensor.matmul(
                        wt[w][:], lhsT=msg[:, ch * 128:(ch + 1) * 128],
                        rhs=oh[:], start=(seen == 0),
                        stop=(seen == sched.win_total[w] - 1))
                    win_seen[w] = seen + 1
            # drain superblock
            for w in range(w0, w1_):
                wdst = min(WIN, NPC - w * WIN)
                zsl = zres[:, w * 128:w * 128 + 128]
                nc.vector.tensor_scalar(
                    out=zsl, in0=wt[w][:], scalar1=b_col[:, 0:1], scalar2=None,
                    op0=Alu.add)
                nc.vector.tensor_reduce(
                    out=stats1[:, w:w + 1], in_=zres[:, w * 128:w * 128 + wdst],
                    axis=mybir.AxisListType.X, op=Alu.add)
                sq = scr.tile([128, 128], f32, tag="sq")
                nc.scalar.activation(
                    out=sq[:, :wdst], in_=zres[:, w * 128:w * 128 + wdst],
                    func=Act.Square, accum_out=stats2[:, w:w + 1])
        win_seen.clear()

    def bn_reduce(lyr):
        """AllReduce stats; returns (a_col, bb_col) affine tiles."""
        s_all = scr.tile([128, 2], f32, tag="sall")
        nc.vector.tensor_reduce(out=s_all[:, 0:1], in_=stats1[:, :NW],
                                axis=mybir.AxisListType.X, op=Alu.add)
        nc.vector.tensor_reduce(out=s_all[:, 1:2], in_=stats2[:, :NW],
                                axis=mybir.AxisListType.X, op=Alu.add)
        nc.sync.dma_start(bn_in[:, :], s_all[:, :])
        nc.gpsimd.collective_compute(
            "AllReduce", Alu.add,
            replica_groups=[list(range(cfg.CORES))],
            ins=[bn_in[:, :]], outs=[bn_out[lyr - 1, :, :]])
        st = colp.tile([128, 2], f32, tag=f"bnst{lyr}")
        nc.sync.dma_start(st[:, :], bn_out[lyr - 1, :, :])
        mu = colp.tile([128, 1], f32, tag=f"mu{lyr}")
        nc.vector.tensor_scalar(out=mu[:], in0=st[:, 0:1], scalar1=1.0 / N,
                                scalar2=None, op0=Alu.mult)
        e2 = colp.tile([128, 1], f32, tag=f"e2{lyr}")
        nc.vector.tensor_scalar(out=e2[:], in0=st[:, 1:2], scalar1=1.0 / N,
                                scalar2=None, op0=Alu.mult)
        var = colp.tile([128, 1], f32, tag=f"var{lyr}")
        nc.vector.tensor_tensor(out=var[:], in0=mu[:], in1=mu[:], op=Alu.mult)
        nc.vector.tensor_tensor(out=var[:], in0=e2[:], in1=var[:],
                                op=Alu.subtract)
        nc.vector.tensor_scalar(out=var[:], in0=var[:], scalar1=cfg.BN_EPS,
                                scalar2=None, op0=Alu.add)
        inv = colp.tile([128, 1], f32, tag=f"inv{lyr}")
        nc.vector.reciprocal(out=inv[:], in_=var[:])
        rstd = colp.tile([128, 1], f32, tag=f"rstd{lyr}")
        nc.scalar.sqrt(out=rstd[:], in_=inv[:])
        a = colp.tile([128, 1], f32, tag=f"a{lyr}")
        nc.vector.tensor_tensor(out=a[:], in0=cols[f"g{lyr}"][:], in1=rstd[:],
                                op=Alu.mult)
        bb = colp.tile([128, 1], f32, tag=f"bb{lyr}")
        nc.vector.tensor_tensor(out=bb[:], in0=mu[:], in1=a[:], op=Alu.mult)
        nc.vector.tensor_tensor(out=bb[:], in0=cols[f"be{lyr}"][:], in1=bb[:],
                                op=Alu.subtract)
        return a, bb

    import os as _os
    _stage = _os.environ.get("GNN_STAGE", "full")
    _reps = int(_os.environ.get("GNN_REPS", "1"))
    _comp = _os.environ.get("GNN_COMP", "")
    if _comp:
        # amplification experiment: repeat one component _reps times
        def gather_variant(table, mode):
            """mode: none|plain|tr|big|sb — idx DMA always included."""
            if mode == "sb":
                TPR = 128
                tbl = const.tile([128, TPR * 128], TDT, tag="sbtable")
                nc.sync.dma_start(tbl[:, :], table[0:TPR * 128, :]
                                  .rearrange("(p t) e -> p (t e)", p=128))
            for sb in range(NSB):
                for sg in range(NSEG):
                    nch = sched.n_chunks_call[(sb, sg)]
                    if nch == 0:
                        continue
                    off = sched.chunk_off[(sb, sg)]
                    it = idxp.tile([128, 8 * CHMAX], i16, tag="it")
                    nc.sync.dma_start(it[:, :8 * nch],
                                      meta[:, off * 8:(off + nch) * 8])
                    if mode == "none":
                        continue
                    msg = msgp.tile([128, CHMAX * 128], TDT, tag="msg")
                    if mode == "plain":
                        mview = msg[:, :nch * 128].rearrange(
                            "p (c e) -> p c e", e=128)
                        gi = nc.gpsimd.dma_gather(
                            out_ap=mview,
                            in_ap=table[sg * SEG:(sg + 1) * SEG, :],
                            idxs_ap=it[:, :8 * nch], num_idxs=nch * 128,
                            num_idxs_reg=nidx_reg(nch * 128), elem_size=128,
                            single_packet=False)
                    elif mode == "sp":
                        mview = msg[:, :nch * 128].rearrange(
                            "p (c e) -> p c e", e=128)
                        gi = nc.gpsimd.dma_gather(
                            out_ap=mview,
                            in_ap=table[sg * SEG:(sg + 1) * SEG, :],
                            idxs_ap=it[:, :8 * nch], num_idxs=nch * 128,
                            num_idxs_reg=nidx_reg(nch * 128), elem_size=128,
                            single_packet=True)
                    elif mode == "big":
                        mview = msg[:, :nch * 128].rearrange(
                            "p (c e) -> p c e", e=256)
                        gi = nc.gpsimd.dma_gather(
                            out_ap=mview,
                            in_ap=table[sg * SEG:(sg + 1) * SEG, :]
                            .rearrange("(a b) e -> a (b e)", b=2),
                            idxs_ap=it[:, :4 * nch], num_idxs=nch * 64,
                            num_idxs_reg=nidx_reg(nch * 64), elem_size=256,
                            single_packet=False)
                    elif mode == "tr":
                        mview = msg[:, :nch * 128].rearrange(
                            "p (c e) -> p c e", c=1)
                        gi = nc.gpsimd.dma_gather(
                            out_ap=mview,
                            in_ap=table[sg * SEG:(sg + 1) * SEG, :],
                            idxs_ap=it[:, :8 * nch], num_idxs=nch * 128,
                            num_idxs_reg=nidx_reg(nch * 128), elem_size=128,
                            transpose=True, single_packet=False)
                    elif mode == "sb":
                        TPR = 128
                        itm = idxp.tile([128, 8 * CHMAX], i16, tag="itm")
                        nc.vector.tensor_scalar(
                            out=itm[:, :8 * nch], in0=it[:, :8 * nch],
                            scalar1=16383, scalar2=None,
                            op0=Alu.bitwise_and)
                        mview = msg[:, :nch * 128].rearrange(
                            "p (c e) -> p c e", c=1)
                        gi = nc.gpsimd.dma_gather(
                            out_ap=mview,
                            in_ap=tbl[:, :],
                            idxs_ap=itm[:, :8 * nch], num_idxs=nch * 128,
                            num_idxs_reg=nidx_reg(nch * 128), elem_size=128,
                            transpose=True, single_packet=False,
                            sbuf_tokens_per_rank=TPR,
                            sbuf_free_dim_per_rank=TPR * 256)
                    _add_dep_helper(gi.ins, lib_inst.ins, sync=False,
                                    reason="gpsimd library order")

        def gather_only(table):
            for sb in range(NSB):
                for sg in range(NSEG):
                    nch = sched.n_chunks_call[(sb, sg)]
                    if nch == 0:
                        continue
                    off = sched.chunk_off[(sb, sg)]
                    it = idxp.tile([128, 8 * CHMAX], i16, tag="it")
                    nc.sync.dma_start(it[:, :8 * nch],
                                      meta[:, off * 8:(off + nch) * 8])
                    msg = msgp.tile([128, CHMAX * 128], TDT, tag="msg")
                    mview = msg[:, :nch * 128].rearrange("p (c e) -> p c e", e=128)
                    gi = nc.gpsimd.dma_gather(
                        out_ap=mview, in_ap=table[sg * SEG:(sg + 1) * SEG, :],
                        idxs_ap=it[:, :8 * nch], num_idxs=nch * 128,
                        num_idxs_reg=nidx_reg(nch * 128), elem_size=128,
                        single_packet=False)
                    _add_dep_helper(gi.ins, lib_inst.ins, sync=False,
                                    reason="gpsimd library order")

        def onehot_only():
            for sb in range(NSB):
                for sg in range(NSEG):
                    nsv = sched.n_sv_call[(sb, sg)]
                    if nsv == 0:
                        continue
                    soff = sched.sv_off[(sb, sg)]
                    sv = svp.tile([128, 2 * SVMAX], f32, tag="sv")
                    nc.sync.dma_start(sv[:, :2 * nsv],
                                      slotval[:, soff * 2:(soff + nsv) * 2])
                    for ci in range(nsv):
                        oh = ohp.tile([128, 128], TDT, tag="oh")
                        nc.vector.tensor_scalar(
                            out=oh[:], in0=iota_row_h[:],
                            scalar1=sv[:, 2 * ci:2 * ci + 1],
                            scalar2=sv[:, 2 * ci + 1:2 * ci + 2],
                            op0=Alu.is_equal, op1=Alu.mult)

        for _r in range(_reps):
            if _comp == "gather":
                gather_only(h1_full)
            elif _comp.startswith("gath"):
                gather_variant(h1_full, _comp[4:])
            elif _comp == "onehot":
                onehot_only()
            elif _comp == "edge":
                edge_layer(1, h1_full)
            elif _comp == "ag":
                for q in range(NSEG):
                    nc.gpsimd.collective_compute(
                        "AllGather", Alu.bypass,
                        replica_groups=[list(range(cfg.CORES))],
                        ins=[h_local[q * PIECE:(q + 1) * PIECE, :]],
                        outs=[h1_full[q * SEG:(q + 1) * SEG, :]])
        return
    if _stage == "a":
        return
    # ======== layer 1 ========
    edge_layer(1, h1_full)
    if _stage == "l1":
        return
    a1, bb1 = bn_reduce(1)
    if _stage == "bn1":
        return
    for w in range(NW):
        zsl = zres[:, w * 128:(w + 1) * 128]
        nc.scalar.activation(out=zsl, in_=zsl, func=Act.Relu,
                             scale=a1[:, 0:1], bias=bb1[:, 0:1])

    # h2_local = z1 @ W2 (z1 channel-major resident) -> node-major DRAM
    for c0 in range(0, NWPAD, 512):
        cw = min(512, NWPAD - c0)
        hp = psum_misc.tile([128, 512], f32, tag="pm")
        nc.tensor.matmul(hp[:, :cw], lhsT=w2t[:], rhs=zres[:, c0:c0 + cw],
                         start=True, stop=True)
        hsb = work.tile([128, 512], f32, tag="h2s")
        nc.vector.tensor_copy(out=hsb[:, :cw], in_=hp[:, :cw])
        whole = min(cw, NPC - c0) == cw and cw == 512
        h2st = work.tile([128, 512], TDT, tag="h2st")
        kful = 0
        for j0 in range(0, cw, 128):
            n0 = c0 + j0
            cnt = min(128, NPC - n0)
            if cnt <= 0:
                break
            tp = psum_misc.tile([128, 512], f32, tag="pm")
            nc.tensor.transpose(tp[:, :128], hsb[:, j0:j0 + 128], ident[:])
            if whole:
                nc.vector.tensor_copy(out=h2st[:, j0:j0 + 128],
                                      in_=tp[:, :128])
                kful += 1
            else:
                ts_ = work.tile([128, 128], TDT, tag="tnmh")
                nc.vector.tensor_copy(out=ts_[:cnt, :], in_=tp[:cnt, :128])
                nc.sync.dma_start(h2_local[n0:n0 + cnt, :], ts_[:cnt, :])
        if whole:
            nc.sync.dma_start(
                h2_local[c0:c0 + 512, :]
                .rearrange("(k p) c -> p k c", p=128),
                h2st[:, :512].rearrange("p (k c) -> p k c", c=128))

    if cfg.PIECES:
        for q in range(NSEG):
            nc.gpsimd.collective_compute(
                "AllGather", Alu.bypass,
                replica_groups=[list(range(cfg.CORES))],
                ins=[h2_local[q * PIECE:(q + 1) * PIECE, :]],
                outs=[h2_full[q * SEG:(q + 1) * SEG, :]])
    else:
        nc.gpsimd.collective_compute(
            "AllGather", Alu.bypass,
            replica_groups=[list(range(cfg.CORES))],
            ins=[h2_local[:, :]], outs=[h2_full[:, :]])

    if _stage == "h2":
        return
    # ======== layer 2 ========
    edge_layer(2, h2_full)
    a2, bb2 = bn_reduce(2)
    YB = 8
    for g0 in range(0, NW, YB):
        kcnt = min(YB, NW - g0)
        full = (g0 + kcnt) * 128 <= NPC
        ys = work.tile([128, YB * 128], f32, tag="ys")
        for k in range(kcnt):
            w = g0 + k
            wdst = min(WIN, NPC - w * WIN)
            if wdst <= 0:
                kcnt = k
                break
            ocm = work.tile([128, 128], f32, tag="ocm")
            nc.vector.tensor_scalar(
                out=ocm[:], in0=zres[:, w * 128:(w + 1) * 128],
                scalar1=a2[:, 0:1], scalar2=bb2[:, 0:1],
                op0=Alu.mult, op1=Alu.add)
            tp = psum_misc.tile([128, 512], f32, tag="pm")
            nc.tensor.transpose(tp[:, :128], ocm[:, :], ident[:])
            if full:
                nc.vector.tensor_copy(out=ys[:, k * 128:(k + 1) * 128],
                                      in_=tp[:, :128])
            else:
                ts_ = work.tile([128, 128], f32, tag="tnm")
                nc.vector.tensor_copy(out=ts_[:wdst, :], in_=tp[:wdst, :128])
                nc.sync.dma_start(y[w * 128:w * 128 + wdst, :], ts_[:wdst, :])
        if full and kcnt:
            nc.sync.dma_start(
                y[g0 * 128:(g0 + kcnt) * 128, :]
                .rearrange("(k p) c -> p k c", p=128),
                ys[:, :kcnt * 128].rearrange("p (k c) -> p k c", c=128))


def build_program(cfg: Cfg, sched: Sched):
    """Create Bass program; returns (nc, input names)."""
    import concourse.bacc as bacc
    import concourse.mybir as mybir
    from concourse.tile import TileContext
    _apply_tile_patch()

    f32 = mybir.dt.float32
    nc = bacc.Bacc(num_devices=cfg.CORES)
    CT = sched.CT
    import os as _os
    if _os.environ.get("GNN_TINY"):
        # shrink all big external tensors to probe host<->device transfer cost
        tensors = {
            "x_shard": nc.dram_tensor("x_shard", [128, cfg.D], f32,
                                      kind="ExternalInput"),
            "consts": nc.dram_tensor("consts", [128, 129], f32,
                                     kind="ExternalInput"),
            "meta": nc.dram_tensor("meta", [128, 8], mybir.dt.int16,
                                   kind="ExternalInput"),
            "slotval": nc.dram_tensor("slotval", [128, 2], f32,
                                      kind="ExternalInput"),
            "W1": nc.dram_tensor("W1", [128, 128], f32, kind="ExternalInput"),
            "W2": nc.dram_tensor("W2", [128, 128], f32, kind="ExternalInput"),
            "b1": nc.dram_tensor("b1", [128, 1], f32, kind="ExternalInput"),
            "g1": nc.dram_tensor("g1", [128, 1], f32, kind="ExternalInput"),
            "be1": nc.dram_tensor("be1", [128, 1], f32, kind="ExternalInput"),
            "b2": nc.dram_tensor("b2", [128, 1], f32, kind="ExternalInput"),
            "g2": nc.dram_tensor("g2", [128, 1], f32, kind="ExternalInput"),
            "be2": nc.dram_tensor("be2", [128, 1], f32, kind="ExternalInput"),
            "y": nc.dram_tensor("y", [128, cfg.D], f32, kind="ExternalOutput"),
        }
        with TileContext(nc) as tc:
            import concourse.mybir as _mb
            with tc.tile_pool(name="tiny", bufs=1) as t:
                tt = t.tile([128, 129], f32)
                nc.sync.dma_start(tt[:], tensors["consts"][:, :])
                ty = t.tile([128, cfg.D], f32)
                nc.vector.tensor_scalar(out=ty[:], in0=tt[:, :128], scalar1=2.0,
                                        scalar2=None, op0=_mb.AluOpType.mult)
                nc.sync.dma_start(tensors["y"][:, :], ty[:])
        if not nc.is_finalized():
            nc.finalize()
        return nc
    tensors = {
        "x_shard": nc.dram_tensor("x_shard", [cfg.D, cfg.NPC], f32,
                                  kind="ExternalInput"),
        "consts": nc.dram_tensor("consts", [128, 129], f32,
                                 kind="ExternalInput"),
        "meta": nc.dram_tensor("meta", [128, 8 * CT], mybir.dt.int16,
                               kind="ExternalInput"),
        "slotval": nc.dram_tensor("slotval", [128, 2 * sched.SVC], f32,
                                  kind="ExternalInput"),
        "W1": nc.dram_tensor("W1", [128, 128], f32, kind="ExternalInput"),
        "W2": nc.dram_tensor("W2", [128, 128], f32, kind="ExternalInput"),
        "b1": nc.dram_tensor("b1", [128, 1], f32, kind="ExternalInput"),
        "g1": nc.dram_tensor("g1", [128, 1], f32, kind="ExternalInput"),
        "be1": nc.dram_tensor("be1", [128, 1], f32, kind="ExternalInput"),
        "b2": nc.dram_tensor("b2", [128, 1], f32, kind="ExternalInput"),
        "g2": nc.dram_tensor("g2", [128, 1], f32, kind="ExternalInput"),
        "be2": nc.dram_tensor("be2", [128, 1], f32, kind="ExternalInput"),
        "y": nc.dram_tensor("y", [cfg.NPC, cfg.D], f32, kind="ExternalOutput"),
    }
    with TileContext(nc) as tc:
        build(nc, tc, cfg, sched, tensors)
        tc._gnn_exitstack.close()
    if not nc.is_finalized():
        nc.finalize()
    return nc


def make_consts():
    c = np.zeros((128, 129), np.float32)
    c[:, :128] = np.arange(128, dtype=np.float32)[None, :]
    c[:, 128] = np.arange(128, dtype=np.float32)
    return c


def kernel_run(inputs: dict, cfg: Cfg):
    """Full flow: prep -> build -> run on 8 cores -> assemble output."""
    import numpy as np
    from concourse.bass_utils import run_bass_kernel_spmd

    x = np.asarray(inputs["x"], np.float32)
    ei = np.asarray(inputs["edge_index"])
    ew = np.asarray(inputs["edge_weight"], np.float32)
    sched, per_core = prep(ei, ew, cfg)
    nc = build_program(cfg, sched)

    com = {
        "W1": np.ascontiguousarray(inputs["W1"], dtype=np.float32),
        "W2": np.ascontiguousarray(inputs["W2"], dtype=np.float32),
        "consts": make_consts(),
    }
    for nm in ("b1", "g1", "be1", "b2", "g2", "be2"):
        com[nm] = np.ascontiguousarray(
            np.asarray(inputs[nm], np.float32).reshape(128, 1))
    in_maps = []
    for ci in range(cfg.CORES):
        m = dict(com)
        m["x_shard"] = np.ascontiguousarray(
            x[ci * cfg.NPC:(ci + 1) * cfg.NPC].T)
        m["meta"] = per_core[ci]["meta"]
        m["slotval"] = per_core[ci]["slotval"]
        in_maps.append(m)
    res = run_bass_kernel_spmd(nc, in_maps, core_ids=list(range(cfg.CORES)))
    out = np.concatenate([r["y"] for r in res.results], axis=0)
    return out, res


def kernel(**inputs) -> np.ndarray:
    cfg = Cfg()
    out, _ = kernel_run(inputs, cfg)
    return out

